# revision 2
# baseline (speedup 1.0000x reference)
"""GatedAttentionPooling Trainium2 kernel.

pooled[b] = sum_i softmax_within_segment((tanh(x V^T) * sigmoid(x U^T)) w^T)[i] * x[i]

Sharding: 8 segments per core (segment-aligned row blocks), no collectives.
Phase 1 computes attention logits A for all rows (bf16 matmuls, transposed
x layout so F is the contraction/partition dim), phase 2 computes
exp(A)-weighted segment sums of x via float32r matmuls with the per-row
weight/one-hot matrix as the stationary operand.
"""

import time

import ml_dtypes
import numpy as np

import concourse.bass as bass
import concourse.mybir as mybir
import concourse.tile as tile
from concourse import bacc
from concourse.bass import ts
from concourse.bass_utils import run_bass_kernel_spmd

N_CORES = 8
F = 512
ADIM = 256
B_TOTAL = 64
SEGS = B_TOTAL // N_CORES  # segments per core

BF16 = mybir.dt.bfloat16
F32 = mybir.dt.float32
F32R = mybir.dt.float32r
AF = mybir.ActivationFunctionType

_BUILD_CACHE: dict[int, object] = {}
LAST_STATS: dict = {}


def _build(pad: int):
    """Build + compile the SPMD module for a per-core row capacity `pad`."""
    t1 = pad // 512   # phase-1 tiles (512 rows each)
    c = pad // 128    # phase-2 chunks (128 rows each)
    kc_n = F // 128   # f-chunks (4)
    m_n = ADIM // 128  # adim chunks (2)

    nc = bacc.Bacc("TRN2", target_bir_lowering=False, debug=False,
                   num_devices=N_CORES)

    xt_d = nc.declare_dram_parameter("xt", [F, pad], BF16, isOutput=False)
    x_d = nc.declare_dram_parameter("x", [pad, F], F32, isOutput=False)
    s_d = nc.declare_dram_parameter("s", [pad, SEGS], F32, isOutput=False)
    vw_d = nc.declare_dram_parameter("vwT", [128, kc_n, ADIM], BF16, isOutput=False)
    uw_d = nc.declare_dram_parameter("uwT", [128, kc_n, ADIM], BF16, isOutput=False)
    ww_d = nc.declare_dram_parameter("wwT", [128, m_n], BF16, isOutput=False)
    vb_d = nc.declare_dram_parameter("vb", [128, m_n], F32, isOutput=False)
    ub_d = nc.declare_dram_parameter("ub", [128, m_n], F32, isOutput=False)
    out_d = nc.declare_dram_parameter("out", [SEGS, F], F32, isOutput=True)

    with tile.TileContext(nc) as tc:
        with (
            tc.tile_pool(name="const", bufs=1) as const_pool,
            tc.tile_pool(name="xin", bufs=3) as x_pool,
            tc.tile_pool(name="act", bufs=2) as act_pool,
            tc.tile_pool(name="avec", bufs=2) as a_pool,
            tc.tile_pool(name="x2", bufs=4) as x2_pool,
            tc.tile_pool(name="wmat", bufs=4) as w_pool,
            tc.tile_pool(name="fin", bufs=1) as fin_pool,
            tc.tile_pool(name="psv", bufs=1, space="PSUM") as psv_pool,
            tc.tile_pool(name="psa", bufs=2, space="PSUM") as psa_pool,
            tc.tile_pool(name="psp", bufs=1, space="PSUM") as psp_pool,
            tc.tile_pool(name="dram", bufs=1, space="DRAM") as dram_pool,
        ):
            # constants
            vw_sb = const_pool.tile([128, kc_n, ADIM], BF16)
            nc.sync.dma_start(out=vw_sb, in_=vw_d[:, :, :])
            uw_sb = const_pool.tile([128, kc_n, ADIM], BF16)
            nc.sync.dma_start(out=uw_sb, in_=uw_d[:, :, :])
            ww_sb = const_pool.tile([128, m_n], BF16)
            nc.sync.dma_start(out=ww_sb, in_=ww_d[:, :])
            vb_sb = const_pool.tile([128, m_n], F32)
            nc.sync.dma_start(out=vb_sb, in_=vb_d[:, :])
            ub_sb = const_pool.tile([128, m_n], F32)
            nc.sync.dma_start(out=ub_sb, in_=ub_d[:, :])
            ones_sb = const_pool.tile([128, 1], F32)
            nc.vector.memset(ones_sb, 1.0)
            s_sb = const_pool.tile([128, c, SEGS], F32)
            nc.sync.dma_start(out=s_sb,
                              in_=s_d[:, :].rearrange("(p c) s -> p c s", p=128))

            a_scr = dram_pool.tile([t1, 512], F32)

            xt_r = xt_d[:, :].rearrange("(kc p) n -> p kc n", p=128)

            # ---- phase 1: logits ----
            for t in range(t1):
                xt_sb = x_pool.tile([128, kc_n, 512], BF16)
                nc.sync.dma_start(out=xt_sb, in_=xt_r[:, :, ts(t, 512)])

                v_ps = psv_pool.tile([128, m_n, 512], F32, tag="v_ps")
                u_ps = psv_pool.tile([128, m_n, 512], F32, tag="u_ps")
                for m in range(m_n):
                    for kc in range(kc_n):
                        nc.tensor.matmul(
                            v_ps[:, m, :],
                            lhsT=vw_sb[:, kc, ts(m, 128)],
                            rhs=xt_sb[:, kc, :],
                            start=(kc == 0), stop=(kc == kc_n - 1),
                        )
                for m in range(m_n):
                    for kc in range(kc_n):
                        nc.tensor.matmul(
                            u_ps[:, m, :],
                            lhsT=uw_sb[:, kc, ts(m, 128)],
                            rhs=xt_sb[:, kc, :],
                            start=(kc == 0), stop=(kc == kc_n - 1),
                        )

                v_sb = act_pool.tile([128, m_n, 512], BF16, tag="v_sb")
                u_sb = act_pool.tile([128, m_n, 512], BF16, tag="u_sb")
                for m in range(m_n):
                    nc.scalar.activation(v_sb[:, m, :], v_ps[:, m, :], AF.Tanh,
                                         bias=vb_sb[:, m:m + 1])
                    nc.scalar.activation(u_sb[:, m, :], u_ps[:, m, :], AF.Sigmoid,
                                         bias=ub_sb[:, m:m + 1])

                vu_sb = act_pool.tile([128, m_n, 512], BF16, tag="vu_sb")
                for m in range(m_n):
                    nc.vector.tensor_mul(vu_sb[:, m, :], v_sb[:, m, :], u_sb[:, m, :])

                a_ps = psa_pool.tile([1, 512], F32)
                for m in range(m_n):
                    nc.tensor.matmul(
                        a_ps, lhsT=ww_sb[:, m:m + 1], rhs=vu_sb[:, m, :],
                        start=(m == 0), stop=(m == m_n - 1),
                    )
                a_sb = a_pool.tile([1, 512], F32)
                nc.vector.tensor_copy(a_sb, a_ps)
                nc.sync.dma_start(out=a_scr[t, :], in_=a_sb)

            # ---- phase 2: exp + weighted segment sums ----
            av_sb = fin_pool.tile([128, c], F32)
            nc.sync.dma_start(
                out=av_sb,
                in_=a_scr[:, :].rearrange("t n -> (t n)").rearrange(
                    "(p c) -> p c", p=128),
            )
            e_sb = fin_pool.tile([128, c], F32)
            nc.scalar.activation(e_sb, av_sb, AF.Exp)

            x2_r = x_d[:, :].rearrange("(p c) f -> p c f", p=128)

            pool_ps = psp_pool.tile([SEGS, F], F32)
            sums_ps = psp_pool.tile([SEGS, 1], F32)
            for j in range(c):
                w_sb = w_pool.tile([128, SEGS], F32)
                nc.vector.tensor_scalar_mul(w_sb, s_sb[:, j, :], e_sb[:, j:j + 1])
                xj_sb = x2_pool.tile([128, F], F32)
                nc.sync.dma_start(out=xj_sb, in_=x2_r[:, j, :])
                nc.tensor.matmul(
                    pool_ps, lhsT=w_sb[:], rhs=xj_sb[:],
                    start=(j == 0), stop=(j == c - 1), skip_group_check=True,
                )
                nc.tensor.matmul(
                    sums_ps, lhsT=w_sb[:], rhs=ones_sb[:],
                    start=(j == 0), stop=(j == c - 1), skip_group_check=True,
                )

            pooled_sb = fin_pool.tile([SEGS, F], F32)
            nc.vector.tensor_copy(pooled_sb, pool_ps)
            sums_sb = fin_pool.tile([SEGS, 1], F32)
            nc.vector.tensor_scalar_add(sums_sb, sums_ps, 1e-9)
            rec_sb = fin_pool.tile([SEGS, 1], F32)
            nc.vector.reciprocal(rec_sb, sums_sb)
            o_sb = fin_pool.tile([SEGS, F], F32)
            nc.vector.tensor_scalar_mul(o_sb, pooled_sb, rec_sb[:, 0:1])
            nc.sync.dma_start(out=out_d[:, :], in_=o_sb)

    nc.compile()
    return nc


def kernel(x, batch_indices, batch_size, Vw, Vb, Uw, Ub, ww, wb, **_kw):
    t_host0 = time.monotonic()
    x = np.asarray(x, dtype=np.float32)
    seg = np.asarray(batch_indices).astype(np.int64)
    n_total = x.shape[0]
    assert int(batch_size) == B_TOTAL and x.shape[1] == F

    Vw = np.asarray(Vw, dtype=np.float32)
    Uw = np.asarray(Uw, dtype=np.float32)
    Vb = np.asarray(Vb, dtype=np.float32)
    Ub = np.asarray(Ub, dtype=np.float32)
    ww = np.asarray(ww, dtype=np.float32).reshape(ADIM)
    # wb drops out of the segment softmax (constant shift per segment).

    bounds = np.searchsorted(seg, np.arange(B_TOTAL + 1))
    core_lo = bounds[0:B_TOTAL + 1:SEGS]  # [9] row starts per core boundary
    n_rows = core_lo[1:] - core_lo[:-1]
    pad = int(((n_rows.max() + 511) // 512) * 512)
    pad = max(pad, 512)

    # replicated weights, in the exact SBUF layouts the kernel loads
    kc_n = F // 128
    m_n = ADIM // 128
    vwT = np.ascontiguousarray(
        Vw.T.reshape(kc_n, 128, ADIM).transpose(1, 0, 2)).astype(ml_dtypes.bfloat16)
    uwT = np.ascontiguousarray(
        Uw.T.reshape(kc_n, 128, ADIM).transpose(1, 0, 2)).astype(ml_dtypes.bfloat16)
    wwT = np.ascontiguousarray(ww.reshape(m_n, 128).T).astype(ml_dtypes.bfloat16)
    vb_h = np.ascontiguousarray(Vb.reshape(m_n, 128).T).astype(np.float32)
    ub_h = np.ascontiguousarray(Ub.reshape(m_n, 128).T).astype(np.float32)

    in_maps = []
    for cid in range(N_CORES):
        r0, r1 = int(core_lo[cid]), int(core_lo[cid + 1])
        n_c = r1 - r0
        xs = x[r0:r1]
        xt = np.zeros((F, pad), dtype=ml_dtypes.bfloat16)
        xt[:, :n_c] = xs.astype(ml_dtypes.bfloat16).T
        xf = np.zeros((pad, F), dtype=np.float32)
        xf[:n_c] = xs
        s_h = np.zeros((pad, SEGS), dtype=np.float32)
        s_h[np.arange(n_c), seg[r0:r1] - SEGS * cid] = 1.0
        in_maps.append({
            "xt": xt, "x": xf, "s": s_h,
            "vwT": vwT, "uwT": uwT, "wwT": wwT, "vb": vb_h, "ub": ub_h,
        })
    t_host1 = time.monotonic()

    if pad not in _BUILD_CACHE:
        tb0 = time.monotonic()
        _BUILD_CACHE[pad] = _build(pad)
        LAST_STATS["build_s"] = time.monotonic() - tb0
    nc = _BUILD_CACHE[pad]

    t_run0 = time.monotonic()
    res = run_bass_kernel_spmd(nc, in_maps, core_ids=list(range(N_CORES)))
    t_run1 = time.monotonic()

    out = np.concatenate([res.results[cid]["out"] for cid in range(N_CORES)], axis=0)
    LAST_STATS.update(
        host_prep_s=t_host1 - t_host0,
        run_s=t_run1 - t_run0,
        exec_time_ns=res.exec_time_ns,
        pad=pad,
    )
    return out.astype(np.float32)
